# revision 1
# baseline (speedup 1.0000x reference)
"""Corr1d cost-volume kernel for Trainium2 (8 NeuronCores), V2.

corr[b, d, h, x] = sum_c fL[b,c,h,x] * fR[b,c,h,x-d]  for x >= d, else 0.
Shapes: fL, fR = (4, 64, 256, 512) fp32; out = (4, 48, 256, 512) fp32.

Sharding: data-parallel over (batch, h-half): core i handles b = i//2,
h rows [128*(i%2), 128*(i%2)+128).

Per-core pipeline (per quad = 4 h rows):
  - 64-wide x-blocks; per h row, 8 banded matmuls [c=64 -> 64 x, 112 win]
    packed two-blocks-per-psum-tile on partition halves -> [128, 4*112]
    fp32 per row, 4 rows per 4-bank psum quad [128, 2048]
  - ACT: one raw copy psum -> SBUF fp16 [128, 4*448] (no mask)
  - DVE: band mask multiply (const 0/1 tile, 2x fp16) + 48-stride fold
    adds -> rotated-band tile F [128, 4*192] (dense = real output size)
  - 2 output DMAs per quad (768B lines) into a dump tensor
Host: un-rotates the band with a precomputed numpy gather (free) and
assembles the fp32 output. x<d entries are zero via the mask (no valid
source column), matching the reference.
"""
import numpy as np
from contextlib import ExitStack

import concourse.bass as bass
import concourse.tile as tile
import concourse.bacc as bacc
import concourse.mybir as mybir
from concourse import bass_utils
from concourse.ap import AP

B, C, H, W = 4, 64, 256, 512
D = 48
NCORES = 8
HH = H // 2            # h rows per core
NH = 16                # h rows per load batch
NBATCH = HH // NH      # 8
WIN = 112              # rhs window width per 64-block
GB = 64                # x-block width
NBLK = W // GB         # 8 blocks per h row
# window start per block: 64b-47 clipped into [0, W-WIN]
SB = [max(0, min(64 * b - 47, W - WIN)) for b in range(NBLK)]

fp16 = mybir.dt.float16
fp32 = mybir.dt.float32


def _make_mask():
    # mask[p, 112t + j] = 1 iff d = x - SB[b] - j in [0, 48),
    # with p = 64H + u, b = 2t + H, x = 64b + u. Replicated x4 (quad rows).
    base = np.zeros((128, 4 * WIN), dtype=np.float16)
    for p in range(128):
        Hc, u = divmod(p, GB)
        for t in range(4):
            b = 2 * t + Hc
            x = GB * b + u
            for j in range(WIN):
                if 0 <= x - SB[b] - j < D:
                    base[p, WIN * t + j] = 1.0
    return np.tile(base, (1, 4))


def _build_nc():
    nc = bacc.Bacc("TRN2", target_bir_lowering=False, debug=False,
                   num_devices=NCORES)
    fL_d = nc.dram_tensor("fLc", [C, HH, W], fp16, kind="ExternalInput").ap()
    fR_d = nc.dram_tensor("fRc", [C, HH, W], fp16, kind="ExternalInput").ap()
    mask_d = nc.dram_tensor("maskc", [128, 4 * 4 * WIN], fp16,
                            kind="ExternalInput").ap()
    # per load-batch: [p, (hpb, pr, hi, t, j')] -> 6KB DMA lines
    dump_d = nc.dram_tensor("dump", [NBATCH, 128, 16 * 4 * D], fp16,
                            kind="ExternalOutput").ap()

    with tile.TileContext(nc) as tc, ExitStack() as ctx:
        const_pool = ctx.enter_context(tc.tile_pool(name="const", bufs=1))
        in_pool = ctx.enter_context(tc.tile_pool(name="inp", bufs=3))
        raw_pool = ctx.enter_context(tc.tile_pool(name="raw", bufs=3))
        msk_pool = ctx.enter_context(tc.tile_pool(name="msk", bufs=3))
        f_pool = ctx.enter_context(tc.tile_pool(name="fold", bufs=3))
        mm_psum = ctx.enter_context(tc.tile_pool(name="mmps", bufs=2, space="PSUM"))

        mask_t = const_pool.tile([128, 4 * 4 * WIN], fp16)
        nc.scalar.dma_start(mask_t[:], mask_d)

        NHH = NH // 2
        for ib in range(NBATCH):
            h0 = ib * NH
            # h rows h0..h0+7 -> partitions 0:64, h0+8..h0+15 -> 64:128
            fl = in_pool.tile([128, NHH * W], fp16, tag="fl")
            fr = in_pool.tile([128, NHH * W], fp16, tag="fr")
            for half in range(2):
                nc.sync.dma_start(
                    fl[64 * half : 64 * half + 64, :]
                    .rearrange("c (h x) -> c h x", h=NHH),
                    fL_d[:, h0 + NHH * half : h0 + NHH * (half + 1), :],
                )
                nc.gpsimd.dma_start(
                    fr[64 * half : 64 * half + 64, :]
                    .rearrange("c (h x) -> c h x", h=NHH),
                    fR_d[:, h0 + NHH * half : h0 + NHH * (half + 1), :],
                )

            F = f_pool.tile([128, 16 * 4 * D], fp16)
            for hpb in range(4):
                # quad = pairs (2*hpb, 2*hpb+1) x hi in {0,1}; psum row
                # q = 2*pr + hi at cols [512q, 512q+448)
                ps = mm_psum.tile([128, 2048], fp32)
                for pr in range(2):
                    hp = 2 * hpb + pr
                    for t in range(4):
                        for Hc in range(2):
                            b = 2 * t + Hc
                            for hi in range(2):
                                q = 2 * pr + hi
                                nc.tensor.matmul(
                                    ps[64 * Hc : 64 * Hc + 64,
                                       512 * q + WIN * t : 512 * q + WIN * (t + 1)],
                                    fl[64 * hi : 64 * hi + 64,
                                       W * hp + GB * b : W * hp + GB * b + GB],
                                    fr[64 * hi : 64 * hi + 64,
                                       W * hp + SB[b] : W * hp + SB[b] + WIN],
                                    start=True,
                                    stop=True,
                                )
                # raw evacuation (ACT): psum fp32 -> SBUF fp16, no mask
                raw = raw_pool.tile([128, 4 * 4 * WIN], fp16)
                nc.scalar.copy(
                    raw[:].rearrange("p (q c) -> p q c", q=4),
                    ps[:].rearrange("p (q c) -> p q c", q=4)[:, :, 0 : 4 * WIN],
                )
                # band mask (DVE, 2x fp16)
                msk = msk_pool.tile([128, 4 * 4 * WIN], fp16)
                nc.vector.tensor_mul(msk[:], raw[:], mask_t[:])
                # fold 112 -> 48 (exactly one nonzero plane per output col)
                Tv = msk[:].rearrange("p (q t j) -> p q t j", q=4, t=4)
                Fv = (F[:, 4 * 4 * D * hpb : 4 * 4 * D * (hpb + 1)]
                      .rearrange("p (q t j) -> p q t j", q=4, t=4))
                with nc.allow_low_precision(reason="fold adds zeros"):
                    nc.vector.tensor_add(
                        Fv, Tv[:, :, :, 0:D], Tv[:, :, :, D : 2 * D]
                    )
                    nc.vector.tensor_add(
                        Fv[:, :, :, 0 : WIN - 2 * D],
                        Fv[:, :, :, 0 : WIN - 2 * D],
                        Tv[:, :, :, 2 * D : WIN],
                    )
            nc.scalar.dma_start(dump_d[ib], F[:])

    nc.compile()
    return nc


_NC_CACHE = None


def _get_nc():
    global _NC_CACHE
    if _NC_CACHE is None:
        _NC_CACHE = _build_nc()
    return _NC_CACHE


def make_in_maps(fL, fR):
    maskc = _make_mask()
    in_maps = []
    for core in range(NCORES):
        b, half = divmod(core, 2)
        sl = np.s_[b, :, half * HH : half * HH + HH, :]
        in_maps.append({
            "fLc": fL[sl].astype(np.float16),
            "fRc": fR[sl].astype(np.float16),
            "maskc": maskc,
        })
    return in_maps


_GATHER_CACHE = None


def _gather_tables():
    # out[d, h, x] = dump[ib, hp, P[x], 192*hi + CB[d, x]] * VALID[d, x]
    global _GATHER_CACHE
    if _GATHER_CACHE is None:
        xs = np.arange(W)
        ds = np.arange(D)
        bx = xs // GB
        ux = xs % GB
        Px = 64 * (bx % 2) + ux                          # [W]
        jabs = xs[None, :] - np.asarray(SB)[bx][None, :] - ds[:, None]  # [D, W]
        valid = (jabs >= 0) & (jabs < WIN)
        tb = bx // 2
        cb = D * tb[None, :] + np.where(valid, jabs, 0) % D             # [D, W]
        FI = Px[None, :] * (4 * D) + cb                  # [D, W] into [p, 192]
        _GATHER_CACHE = (FI.astype(np.int64), valid.astype(np.float32))
    return _GATHER_CACHE


def kernel(fL: np.ndarray, fR: np.ndarray) -> np.ndarray:
    fL = np.asarray(fL, dtype=np.float32)
    fR = np.asarray(fR, dtype=np.float32)
    nc = _get_nc()
    in_maps = make_in_maps(fL, fR)
    res = bass_utils.run_bass_kernel_spmd(nc, in_maps, core_ids=list(range(NCORES)))

    FI, valid = _gather_tables()
    out = np.empty((B, D, H, W), dtype=np.float32)
    for core in range(NCORES):
        b, half = divmod(core, 2)
        dump = res.results[core]["dump"]                 # [8, 128, 3072]
        # [ib, p, hpb, pr, hi, c] -> [h = (ib, hi, hpb, pr), p*192 + c]
        arr = (dump.reshape(NBATCH, 128, 4, 2, 2, 4 * D)
               .transpose(0, 4, 2, 3, 1, 5)
               .reshape(HH, 128 * 4 * D))
        g = arr[:, FI.reshape(-1)].reshape(HH, D, W).astype(np.float32)
        g *= valid[None, :, :]
        out[b, :, half * HH : half * HH + HH, :] = g.transpose(1, 0, 2)
    return out

